# revision 29
# baseline (speedup 1.0000x reference)
"""Trainium2 Bass kernel for ChunkedLocalSelfAttention.

Module: x[B,C,H,W] -> qkv proj -> 8-head local-window attention (17x17
spatial window) -> out proj -> +residual -> 1x1 conv -> relu.
B,C,H,W = 4,256,48,48; per-core half-image: 1152 queries, head dim 32.

Sharding: 8 cores = 4 batch images x 2 query-row-halves (24 rows each).
Attention rows depend only on +-8 image rows, so a 32-row k/v band per
core needs no cross-core traffic. Half-1 images ship vertically flipped
(window test is |dh|<=8, flip-equivariant) so one SPMD program serves
both halves.

On-core design (W-MAJOR token order: q token = 24*w + h, band token =
32*w + h). W-major makes the fine-grained mask axis the dense 24/32-row
axis (17-wide window ~ 50% dense) and the 48-col axis structurally
trimmed at chunk granularity -> ~29% fewer score elements than h-major.

  - scoresT[key,q] per (qblock of 128 q, chunk of 128 keys = 4 w-cols):
    8 head matmuls, K=32 row-packed via tile_position; q-range trimmed
    to the chunk's +-8 w-col reach (32-aligned for free).
  - exp on ScalarE (scale fused; logits tiny, no max-subtraction), one
    instr per pair covering all 8 heads [128, 8, width].
  - binary 2D window mask multiply on VectorE/GpSimd (host precomputed).
  - FLIPPED PV: masked probs are the STATIONARY operand, [v | 1] the
    moving one -> 33 output cols per (pair, head) instead of width.
    PSUM accumulates [128 q, 8*(32 pv + 1 sum)] per qblock (zero-matmul
    opens the bank).
  - normalize per qblock (recip of sums col, broadcast multiply), PE
    transpose to channel-major, DMA psum->sbuf, then out proj,
    +residual (out_b folded host-side), 1x1 conv, relu+conv_b.
"""

import sys

for _p in ("/opt/trn_rl_repo",):
    if _p not in sys.path:
        sys.path.insert(0, _p)

import math

import ml_dtypes
import numpy as np

B, C, H, W = 4, 256, 48, 48
HEADS, HD, HALF = 8, 32, 8
NCORES = 8
RQ = 24                    # query rows per core
RB = 32                    # k/v band rows (24 + 8 halo)
NQ = W * RQ                # 1152 query tokens (w-major)
NB = W * RB                # 1536 band tokens (w-major)
NQB = NQ // 128            # 9 query blocks
NCK = NB // 128            # 12 key chunks (4 w-cols x 32 h each)
SCALE = 1.0 / math.sqrt(HD)

bf16 = ml_dtypes.bfloat16


def _pairs():
    """(qblock, chunk) pairs with their 32-aligned q-token overlap.

    Chunk ck covers w-cols [4ck, 4ck+4); its +-8 w-window reaches q
    tokens [96ck-192, 96ck+288). 96 and 128 are both multiples of 32,
    so overlap bounds are 32-aligned automatically.
    """
    ps, moff = [], 0
    for qb in range(NQB):
        for ck in range(NCK):
            a = max(128 * qb, 96 * ck - 192)
            b = min(128 * qb + 128, 96 * ck + 288, NQ)
            if a < b:
                ps.append((qb, ck, a, b, moff))
                moff += b - a
    return ps, moff


PAIRS, MTOT = _pairs()


def _pieces(a, w):
    """Split [a, a+w) into base-aligned pieces (PSUM partition rule:
    an access at base b may span at most the aligned block size)."""
    out, b, end = [], a, a + w
    while b < end:
        for s in (128, 64, 32):
            if b % s == 0 and b + s <= end:
                out.append((b, s))
                b += s
                break
        else:
            raise AssertionError((a, w))
    return out


# pairs whose mask-multiply runs on GpSimd instead of DVE (load balance)
POOL_MASK = frozenset(
    i for i in range(len(PAIRS)) if i % 3 == 1 and 2 <= i < len(PAIRS) - 4
)

_PROG = None


def _build_program():
    import concourse.bass as bass  # noqa: F401
    import concourse.mybir as mybir
    import concourse.tile as tile
    from concourse import bacc

    f32 = mybir.dt.float32
    bft = mybir.dt.bfloat16
    AF = mybir.ActivationFunctionType
    OP = mybir.AluOpType

    nc = bacc.Bacc(
        "TRN2", target_bir_lowering=False, debug=False, num_devices=NCORES
    )

    def din(name, shape, dt=bft):
        return nc.dram_tensor(name, shape, dt, kind="ExternalInput").ap()

    xt_d = din("xT", [C, NB])
    wqk_d = din("wqkT", [C, 2 * C])
    wv_d = din("wvT", [C, C])
    wco_d = din("wcoT", [C, C])
    wc_d = din("wcT", [C, C])
    bqk_d = din("bqk", [128, 4], f32)
    bv_d = din("bvrep", [128, C], f32)
    bc_d = din("bcrep", [128, 2], f32)
    mask_d = din("masks", [128, MTOT])
    id_d = din("ident", [128, 128])
    out_d = nc.dram_tensor("out", [C, NQ], f32, kind="ExternalOutput").ap()

    with tile.TileContext(nc) as tc:
        import contextlib

        ctx = contextlib.ExitStack()
        with ctx:
            cpool = ctx.enter_context(tc.tile_pool(name="const", bufs=1))
            qkpool = ctx.enter_context(tc.tile_pool(name="qk", bufs=1))
            vpool = ctx.enter_context(tc.tile_pool(name="v", bufs=1))
            epool = ctx.enter_context(tc.tile_pool(name="exp", bufs=4))
            apool = ctx.enter_context(tc.tile_pool(name="attn", bufs=4))
            rpool = ctx.enter_context(tc.tile_pool(name="recip", bufs=3))
            opool = ctx.enter_context(tc.tile_pool(name="outb", bufs=3))
            psSC = ctx.enter_context(
                tc.tile_pool(name="psSC", bufs=2, space="PSUM")
            )
            psPV = ctx.enter_context(
                tc.tile_pool(name="psPV", bufs=2, space="PSUM")
            )
            psT = ctx.enter_context(
                tc.tile_pool(name="psT", bufs=2, space="PSUM")
            )

            # ---- constants / inputs to SBUF ----
            xt = [cpool.tile([128, NB], bft, tag=f"xt{t}", name=f"xt{t}") for t in range(2)]
            for t in range(2):
                nc.sync.dma_start(xt[t][:], xt_d[128 * t : 128 * t + 128, :])
            wqk = [cpool.tile([128, 2 * C], bft, tag=f"wqk{t}", name=f"wqk{t}") for t in range(2)]
            wv = [cpool.tile([128, C], bft, tag=f"wv{t}", name=f"wv{t}") for t in range(2)]
            wco = [cpool.tile([128, C], bft, tag=f"wco{t}", name=f"wco{t}") for t in range(2)]
            wc = [cpool.tile([128, C], bft, tag=f"wc{t}", name=f"wc{t}") for t in range(2)]
            for t in range(2):
                sl = slice(128 * t, 128 * t + 128)
                nc.sync.dma_start(wqk[t][:], wqk_d[sl, :])
            bqk = cpool.tile([128, 4], f32, tag="bqk")
            bvr = cpool.tile([128, C], f32, tag="bvr")
            bcr = cpool.tile([128, 2], f32, tag="bcr")
            nc.sync.dma_start(bqk[:], bqk_d[:])
            ident = cpool.tile([128, 128], bft, tag="ident")
            nc.sync.dma_start(ident[:], id_d[:])
            zrow = cpool.tile([1, 512], bft, tag="zrow")
            nc.vector.memset(zrow[:], 0.0)
            msk = cpool.tile([128, MTOT], bft, tag="msk")
            nc.sync.dma_start(msk[:], mask_d[:])

            # ---- phase 1: qk projection ----
            # qq[quad] [128, NQ]: q channels of heads 4*quad..4*quad+4,
            # q tokens w-major (strided view of the band).
            # kk[quad] [128, NB]: k channels, all band tokens.
            qq = [qkpool.tile([128, NQ], bft, tag=f"qq{i}", name=f"qq{i}") for i in range(2)]
            kk = [qkpool.tile([128, NB], bft, tag=f"kk{i}", name=f"kk{i}") for i in range(2)]
            vt = [vpool.tile([128, HEADS * 33], bft, tag=f"v{i}", name=f"v{i}") for i in range(NCK)]

            def q_proj(quad, t):
                ps = psSC.tile([128, 1024], f32, tag="sc", name="sc")
                for cc in range(2):
                    rhs = xt[cc][:].rearrange("p (w h) -> p w h", h=RB)[
                        :, 16 * t : 16 * t + 16, 0:RQ
                    ]
                    nc.tensor.matmul(
                        ps[:, 0:384],
                        lhsT=wqk[cc][:, 128 * quad : 128 * quad + 128],
                        rhs=rhs,
                        start=(cc == 0),
                        stop=(cc == 1),
                    )
                dst = qq[quad][:, 384 * t : 384 * t + 384]
                nc.vector.tensor_scalar_add(
                    dst, ps[:, 0:384], bqk[:, quad : quad + 1]
                )

            def k_proj(quad, t):
                ps = psSC.tile([128, 1024], f32, tag="sc", name="sc")
                for cc in range(2):
                    nc.tensor.matmul(
                        ps[:, 0:512],
                        lhsT=wqk[cc][:, 256 + 128 * quad : 256 + 128 * quad + 128],
                        rhs=xt[cc][:, 512 * t : 512 * t + 512],
                        start=(cc == 0),
                        stop=(cc == 1),
                    )
                dst = kk[quad][:, 512 * t : 512 * t + 512]
                nc.vector.tensor_scalar_add(
                    dst, ps[:, 0:512], bqk[:, 2 + quad : 3 + quad]
                )

            def v_proj(i):
                ps = psSC.tile([128, 1024], f32, tag="sc", name="sc")
                for cc in range(2):
                    nc.tensor.matmul(
                        ps[:, 0:C],
                        lhsT=xt[cc][:, 128 * i : 128 * i + 128],
                        rhs=wv[cc][:],
                        start=(cc == 0),
                        stop=(cc == 1),
                    )
                va = vt[i][:].rearrange("p (h x) -> p h x", x=33)
                nc.vector.tensor_add(
                    va[:, :, 0:32],
                    ps[:, 0:C].rearrange("p (h d) -> p h d", d=32),
                    bvr[:].rearrange("p (h d) -> p h d", d=32),
                )
                nc.gpsimd.memset(va[:, :, 32:33], 1.0)

            # All-heads-at-partition-0 copies of q/k (heads along the
            # free dim), filled by partition-shifting DMAs as soon as each
            # quad completes: score matmuls then all run at row group 0,
            # so PSUM banks stay row-group-pure (mixing tile_position row
            # groups within a bank faults) and score tiles rotate freely.
            q32 = qkpool.tile([32, HEADS * NQ], bft, tag="q32")
            k32 = qkpool.tile([32, HEADS * NB], bft, tag="k32")
            q32v = q32[:].rearrange("p (h n) -> p h n", n=NQ)
            k32v = k32[:].rearrange("p (h n) -> p h n", n=NB)

            q_shift_done = set()
            k_shift_done = set()

            def q_shift(t):
                # heads to partition 0 for q tokens [384t, 384t+384)
                if t in q_shift_done:
                    return
                q_shift_done.add(t)
                for h in range(HEADS):
                    quad, st = h // 4, 32 * (h % 4)
                    eng = nc.sync if h % 2 == 0 else nc.gpsimd
                    eng.dma_start(
                        q32v[:, h, 384 * t : 384 * t + 384],
                        qq[quad][st : st + 32, 384 * t : 384 * t + 384],
                    )

            def k_shift(t):
                if t in k_shift_done:
                    return
                k_shift_done.add(t)
                for h in range(HEADS):
                    quad, st = h // 4, 32 * (h % 4)
                    eng = nc.sync if h % 2 == 1 else nc.gpsimd
                    eng.dma_start(
                        k32v[:, h, 512 * t : 512 * t + 512],
                        kk[quad][st : st + 32, 512 * t : 512 * t + 512],
                    )

            for quad in range(2):
                for t in range(3):
                    q_proj(quad, t)
                for st in range(4):
                    eng = nc.sync if st % 2 == 0 else nc.gpsimd
                    eng.dma_start(
                        q32v[:, 4 * quad + st, :],
                        qq[quad][32 * st : 32 * st + 32, :],
                    )
            for t in range(2):
                sl = slice(128 * t, 128 * t + 128)
                nc.sync.dma_start(wv[t][:], wv_d[sl, :])
            nc.sync.dma_start(bvr[:], bv_d[:])
            for quad in range(2):
                for t in range(3):
                    k_proj(quad, t)
                for st in range(4):
                    eng = nc.sync if st % 2 == 1 else nc.gpsimd
                    eng.dma_start(
                        k32v[:, 4 * quad + st, 0:1024],
                        kk[quad][32 * st : 32 * st + 32, 0:1024],
                    )

            def k_shift_rest():
                for quad in range(2):
                    for st in range(4):
                        eng = nc.sync if st % 2 == 1 else nc.gpsimd
                        eng.dma_start(
                            k32v[:, 4 * quad + st, 1024:NB],
                            kk[quad][32 * st : 32 * st + 32, 1024:NB],
                        )
            # output-path loads: needed only at the first projections
            for t in range(2):
                sl = slice(128 * t, 128 * t + 128)
                nc.sync.dma_start(wco[t][:], wco_d[sl, :])
                nc.sync.dma_start(wc[t][:], wc_d[sl, :])
            nc.sync.dma_start(bcr[:], bc_d[:])



            # ---- phase 2: attention ----
            # Score PSUM: ONE [128, 2048] tile = 4 banks, each bank
            # krow-pure (bank b holds heads {b, b+4}, both contract over
            # partition rows [32b, 32b+32)) -- mixing tile_position row
            # groups within a PSUM bank is illegal. Two pair-slots per
            # bank give double buffering: col(h, slot) = 512*(h%4) +
            # 256*slot + 128*(h//4).
            # Emission is software-pipelined for the FIFO engine queues:
            # PV matmuls of pair j are emitted after the scores of pair
            # j+1, and each qblock's normalize/transpose chain two pairs
            # later, so no queued instruction waits long on another
            # engine.
            oT = [cpool.tile([128, NQ], bft, tag=f"oT{g}", name=f"oT{g}") for g in range(2)]

            pv_tiles = {}
            sc_tiles = {}

            def zero_pv(qb):
                pv = psPV.tile([128, 512], f32, tag="pv", name="pv")
                pv_tiles[qb] = pv
                nc.tensor.matmul(
                    pv[:, 0 : HEADS * 33],
                    lhsT=zrow[:, 0:128],
                    rhs=zrow[:, 0 : HEADS * 33],
                    start=True,
                    stop=False,
                    skip_group_check=True,
                )

            def scores(j, qb, ck, a, b):
                aoff = a - 128 * qb
                w_ = b - a
                sc = psSC.tile([128, 1024], f32, tag="sc", name="sc")
                sc_tiles[j] = sc
                for h in range(HEADS):
                    nc.tensor.matmul(
                        sc[:, 128 * h + aoff : 128 * h + aoff + w_],
                        lhsT=k32v[:, h, 128 * ck : 128 * ck + 128],
                        rhs=q32v[:, h, a:b],
                        start=True,
                        stop=True,
                    )

            def exp_mask(j, idx, qb, a, b, moff):
                aoff = a - 128 * qb
                w_ = b - a
                sc = sc_tiles.pop(j)
                sc_v = sc[:].rearrange("p (h q) -> p h q", q=128)[
                    :, :, aoff : aoff + w_
                ]
                ex = epool.tile([128, 1024], bft, tag="ex", name="ex")
                ex_v = ex[:].rearrange("p (h q) -> p h q", q=128)[
                    :, :, aoff : aoff + w_
                ]
                nc.scalar.activation(ex_v, sc_v, AF.Exp, scale=SCALE)
                ma = apool.tile([128, 1024], bft, tag="ma", name="ma")
                ma_v = ma[:].rearrange("p (h q) -> p h q", q=128)[
                    :, :, aoff : aoff + w_
                ]
                mk = msk[:, moff : moff + w_]
                eng = nc.gpsimd if idx in POOL_MASK else nc.vector
                eng.tensor_mul(
                    ma_v,
                    ex_v,
                    mk[:, None, :].broadcast_to([128, HEADS, w_]),
                )
                return ma

            def pv_accum(qb, ck, a, b, ma, last_of_qb):
                aoff = a - 128 * qb
                w_ = b - a
                pv = pv_tiles[qb]
                ma_v = ma[:].rearrange("p (h q) -> p h q", q=128)
                vt_v = vt[ck][:].rearrange("p (h x) -> p h x", x=33)
                pcs = _pieces(aoff, w_)
                for h in range(HEADS):
                    for pi, (pb, pw) in enumerate(pcs):
                        nc.tensor.matmul(
                            pv[pb : pb + pw, 33 * h : 33 * h + 33],
                            lhsT=ma_v[:, h, pb : pb + pw],
                            rhs=vt_v[:, h, :],
                            start=False,
                            stop=(
                                last_of_qb
                                and h == HEADS - 1
                                and pi == len(pcs) - 1
                            ),
                            skip_group_check=True,
                            tile_position=(0, pb),
                        )

            def norm_a(qb):
                # DVE part: normalize into the token-major ot tile
                pv = pv_tiles.pop(qb)
                pv_v = pv[:, 0 : HEADS * 33].rearrange("p (h x) -> p h x", x=33)
                rc = rpool.tile([128, 8], f32, tag="rc", name="rc")
                nc.vector.reciprocal(
                    rc[:].rearrange("p (h x) -> p h x", x=1), pv_v[:, :, 32:33]
                )
                ot = opool.tile([128, C], bft, tag="ot", name="ot")
                nc.vector.tensor_mul(
                    ot[:].rearrange("p (h d) -> p h d", d=32),
                    pv_v[:, :, 0:32],
                    rc[:]
                    .rearrange("p (h x) -> p h x", x=1)
                    .broadcast_to([128, 8, 32]),
                )
                return ot

            def norm_b(qb, ot):
                # PE transposes one slot later so their ldweights never
                # block the PE queue on the DVE normalize
                pst = psT.tile([128, 1024], bft, tag="pst", name="pst")
                for cc in range(2):
                    nc.tensor.transpose(
                        pst[:, 128 * cc : 128 * cc + 128],
                        ot[:, 128 * cc : 128 * cc + 128],
                        ident[:],
                    )
                    nc.vector.tensor_copy(
                        oT[cc][:, 128 * qb : 128 * qb + 128],
                        pst[:, 128 * cc : 128 * cc + 128],
                    )

            def projections(g3, oc):
                # out = relu(Wco.o + Wc.x + bco): the residual rides the
                # conv matmul as two extra accumulation matmuls over the
                # (w-major, strided) q view of the x band -- no extra
                # PSUM drain for the out-projection or residual.
                n0 = 384 * g3
                ps = psPV.tile([128, 512], f32, tag="pv", name="pv")
                for cc in range(2):
                    nc.tensor.matmul(
                        ps[:, 0:384],
                        lhsT=wco[cc][:, 128 * oc : 128 * oc + 128],
                        rhs=oT[cc][:, n0 : n0 + 384],
                        start=(cc == 0),
                        stop=False,
                        skip_group_check=True,
                    )
                for cc in range(2):
                    xq_v = xt[cc][:].rearrange("p (w h) -> p w h", h=RB)[
                        :, 16 * g3 : 16 * g3 + 16, 0:RQ
                    ]
                    nc.tensor.matmul(
                        ps[:, 0:384],
                        lhsT=wc[cc][:, 128 * oc : 128 * oc + 128],
                        rhs=xq_v,
                        start=False,
                        stop=(cc == 1),
                        skip_group_check=True,
                    )
                ob = opool.tile([128, 384], f32, tag="ob", name="ob")
                nc.vector.tensor_scalar(
                    ob[:],
                    ps[:, 0:384],
                    bcr[:, oc : oc + 1],
                    0.0,
                    OP.add,
                    OP.max,
                )
                nc.sync.dma_start(
                    out_d[128 * oc : 128 * oc + 128, n0 : n0 + 384],
                    ob[:],
                )

            # flattened pipelined emission with per-job due slots: PV of
            # a DVE-masked pair is emitted 1 slot later, of a Pool-masked
            # pair 3 slots later (GpSimd mask-mult is ~2.1us -- it must
            # never gate the in-order PE queue); PSUM accumulation order
            # is commutative. stop= goes on the last-EMITTED PV of a
            # qblock; normalize follows one slot after that.
            npair = len(PAIRS)
            pv_due = [
                j + (3 if j in POOL_MASK else (1 if j < npair - 2 else 0))
                for j in range(npair)
            ]
            last_emit = {}  # qb -> j of the latest-due PV
            for j, (qb, ck, a, b, moff) in enumerate(PAIRS):
                if qb not in last_emit or pv_due[j] >= pv_due[last_emit[qb]]:
                    last_emit[qb] = j

            pend = []  # (due, seq, fn) min-heap by (due, seq)
            import heapq

            seq_ctr = [0]

            def push(due, fn):
                heapq.heappush(pend, (due, seq_ctr[0], fn))
                seq_ctr[0] += 1

            def flush(now):
                while pend and pend[0][0] <= now:
                    heapq.heappop(pend)[2]()

            v_done = set()
            for j, (qb, ck, a, b, moff) in enumerate(PAIRS):
                if j == 6:
                    k_shift_rest()
                if ck not in v_done:
                    v_done.add(ck)
                    v_proj(ck)
                if j == 0 or PAIRS[j - 1][0] != qb:
                    zero_pv(qb)
                scores(j, qb, ck, a, b)
                ma = exp_mask(j, j, qb, a, b, moff)
                stop = last_emit[qb] == j
                push(
                    pv_due[j],
                    lambda qb=qb, ck=ck, a=a, b=b, ma=ma, stop=stop: pv_accum(
                        qb, ck, a, b, ma, stop
                    ),
                )
                if PAIRS[min(j, len(PAIRS) - 1)][0] != (
                    PAIRS[j + 1][0] if j + 1 < len(PAIRS) else -1
                ):
                    # last pair of this qblock
                    due_n = pv_due[last_emit[qb]] + 1

                    def norm_chain(qb=qb):
                        ot = norm_a(qb)
                        push(due_n + 1, lambda: norm_b(qb, ot))

                    push(due_n, norm_chain)
                    if qb % 3 == 2:
                        g3 = qb // 3
                        push(due_n + 2, lambda g3=g3: projections(g3, 0))
                        push(due_n + 3, lambda g3=g3: projections(g3, 1))
                flush(j)
            flush(10**9)

    nc.compile()
    return nc


def _get_program():
    global _PROG
    if _PROG is None:
        _PROG = _build_program()
    return _PROG


_MASK_CACHE = {}


def _masks() -> np.ndarray:
    """[128, MTOT] binary window masks, shared by every core.

    Per (qblock, chunk) pair: key partition p -> (w_k, h_k) =
    (4ck + p//32, p%32); q token t -> (t//24, t%24)."""
    if "m" in _MASK_CACHE:
        return _MASK_CACHE["m"]
    m = np.zeros((128, MTOT), bf16)
    p = np.arange(128)
    for qb, ck, a, b, moff in PAIRS:
        wk, hk = 4 * ck + p // 32, p % 32
        t = np.arange(a, b)
        wq, hq = t // RQ, t % RQ
        m[:, moff : moff + b - a] = (
            (np.abs(wk[:, None] - wq[None, :]) <= HALF)
            & (np.abs(hk[:, None] - hq[None, :]) <= HALF)
        ).astype(bf16)
    _MASK_CACHE["m"] = m
    return m


def _prep_core_inputs(core, x, in_proj_w, in_proj_b, out_w, out_b, conv_w, conv_b):
    b, half = core // 2, core % 2
    ximg = x[b]
    if half == 1:
        ximg = ximg[:, ::-1, :]  # row-flip: half-1 becomes half-0 geometry
    band = ximg[:, :RB, :].transpose(0, 2, 1)  # [C, W, RB] w-major
    wco = conv_w @ out_w                       # fused conv(out_proj(.))
    bco = conv_w @ out_b + conv_b
    return {
        "xT": np.ascontiguousarray(band.reshape(C, NB)).astype(bf16),
        "wqkT": np.ascontiguousarray(in_proj_w[: 2 * C].T).astype(bf16),
        "wvT": np.ascontiguousarray(in_proj_w[2 * C :].T).astype(bf16),
        "wcoT": np.ascontiguousarray(wco.T).astype(bf16),
        "wcT": np.ascontiguousarray(conv_w.T).astype(bf16),
        "bqk": np.ascontiguousarray(
            in_proj_b[: 2 * C].reshape(4, 128).T
        ).astype(np.float32),
        "bvrep": np.broadcast_to(in_proj_b[2 * C :], (128, C)).astype(np.float32).copy(),
        "bcrep": np.ascontiguousarray(bco.reshape(2, 128).T).astype(np.float32),
        "masks": _masks(),
        "ident": np.eye(128, dtype=bf16),
    }


def kernel(**inputs):
    from concourse.bass_utils import run_bass_kernel_spmd

    args = {k: np.asarray(v) for k, v in inputs.items()}
    nc = _get_program()
    in_maps = [_prep_core_inputs(core, **args) for core in range(NCORES)]
    res = run_bass_kernel_spmd(nc, in_maps, core_ids=list(range(NCORES)))
    out = np.zeros((B, C, H, W), np.float32)
    for core in range(NCORES):
        b, half = core // 2, core % 2
        o = res.results[core]["out"].reshape(C, W, RQ).transpose(0, 2, 1)
        if half == 1:
            o = o[:, ::-1, :]  # undo the row flip
            out[b][:, RQ:, :] = o
        else:
            out[b][:, :RQ, :] = o
    return out
